# revision 10
# baseline (speedup 1.0000x reference)
"""Bass/Trainium2 kernel for nn_BiHgru2_1d (bidirectional HGRU block), 8-core SPMD.

Math (reference):
    feat = x @ W_in.T + b_in                    # (N,B,3D)
    inp, og, fg = split(feat); inp=silu(inp); og=sigmoid(og); lam=sigmoid(fg)
    u[h,d,e] = (1-lam[h,d]) * inp[h,e];  lam_f[h,d,e] = lam[h,d]
    s = fwd_scan(lam_f, u) + rev_scan(lam_f, u)         # h_t = lam_t h_{t-1} + u_t
    o[h,e] = sum_d s[h,d,e]*og[h,d]; o = LN(o)*gamma+beta; out = o @ W_out.T + b_out

Sharding: 8-way tensor parallel over heads (128 heads/core). Each core:
  GEMM1 (x full; inp path f16, og+fg paths fp8e4m3 DoubleRow) -> activations ->
  per-(b,d,e) tensor_tensor_scan fwd+rev (u reused between directions) ->
  o assembly -> per-batch AllToAll (reshard channel->token) -> LayerNorm fully
  folded into GEMM2 (ot prescaled by gamma*(-rstd), b_t*c1+c2 added as a K=2
  row of the same PSUM accumulation) -> each core writes tokens
  (all b, n in [256i, 256(i+1))) of the output.

Sign trick: we compute u' = (lam-1)*inp = -u, so s' = -s, o' = -o.  With
a_t = -rstd_t and b_t = rstd_t*mu'_t the GEMM2 input ot*gamma*a_t and the
K=2 fold [b_t;1]@[c1;c2] give the exact LN(o)@W2T + c2 despite the sign.

Pass2 is interleaved into pass1 on the tensor queue to keep the PE array
warm (HAM clock gate) and hide the collective latency.
"""

import sys

for _p in ("/opt/trn_rl_repo",):
    if _p not in sys.path:
        sys.path.insert(0, _p)

import numpy as np

# ---- problem constants (hardcoded per contract) ----
N_FULL, B, D = 2048, 4, 2048
E = 2
H = D // E                      # 1024 heads
NCORES = 8
P = 128                         # partitions
HC = H // NCORES                # 128 heads per core
KC = D // P                     # 16 k-chunks
M_TILES = 6                     # [inp e0, inp e1, og e0, og e1, fg d0, fg d1]

_BUILD_CACHE = {}


def build_program(T=N_FULL, num_devices=NCORES, debug_dump=False):
    """Build the SPMD Bass program (same program on every core)."""
    import concourse.bass as bass
    import concourse.mybir as mybir
    import concourse.tile as tile
    from concourse import bacc

    f16 = mybir.dt.float16
    f32 = mybir.dt.float32
    fp8 = mybir.dt.float8e4
    MUL = mybir.AluOpType.mult
    ADD = mybir.AluOpType.add
    SUB = mybir.AluOpType.subtract
    AF = mybir.ActivationFunctionType
    DR = mybir.MatmulPerfMode.DoubleRow

    NSEG = T // NCORES           # per-core seq positions per batch (256)
    TOK_C = B * NSEG             # tokens per core after reshard (1024)
    NBLK = min(512, T)           # GEMM1 token-block size (per batch)
    NB1 = T // NBLK              # token blocks per batch (4)
    TCH = min(P, NSEG)           # GEMM2 token-chunk (output partition dim)
    NTCH = NSEG // TCH           # token chunks per round (2)
    OCB = 512                    # GEMM2 out-col block
    NOC = D // OCB
    KD = KC // 2                 # DoubleRow k-chunk pairs (8)
    assert T % (NCORES * TCH) == 0 and T % NBLK == 0

    nc = bacc.Bacc("TRN2", target_bir_lowering=False, debug=False,
                   num_devices=num_devices)

    # ---- per-core DRAM parameters ----
    xT_d = nc.dram_tensor("xT", [D, B * T], f16, kind="ExternalInput")
    xT8_d = nc.dram_tensor("xT8", [D, B * T], fp8, kind="ExternalInput")
    w1T_d = nc.dram_tensor("w1T", [D, 2 * P], f16, kind="ExternalInput")
    w18_d = nc.dram_tensor("w18", [D, 4 * P], fp8, kind="ExternalInput")
    b1_d = nc.dram_tensor("b1", [P, M_TILES], f32, kind="ExternalInput")
    w2T_d = nc.dram_tensor("w2T", [D, D], f16, kind="ExternalInput")
    gam_d = nc.dram_tensor("gam", [P, KC], f32, kind="ExternalInput")
    c12_d = nc.dram_tensor("c12", [2, D], f16, kind="ExternalInput")
    out_d = nc.dram_tensor("out", [TOK_C, D], f32, kind="ExternalOutput")
    if debug_dump:
        dbg_oac = nc.dram_tensor("dbg_oac", [B, P, E, T], f16, kind="ExternalOutput")
        dbg_ot = nc.dram_tensor("dbg_ot", [B, P, KC, NSEG], f16, kind="ExternalOutput")

    xT_r = xT_d.ap().rearrange("(kc p) t -> p kc t", p=P)
    xT8_r = xT8_d.ap().rearrange("(kd ko p) t -> p kd ko t", p=P, ko=2)
    w1T_r = w1T_d.ap().rearrange("(kc p) m -> p kc m", p=P)
    w18_r = w18_d.ap().rearrange("(kd ko p) m -> p kd ko m", p=P, ko=2)
    w2T_r = w2T_d.ap().rearrange("(kc p) o -> p kc o", p=P)

    with tile.TileContext(nc) as tc:
        with (
            tc.tile_pool(name="cst", bufs=1) as cst_pool,
            tc.tile_pool(name="w1p", bufs=1) as w1_pool,
            tc.tile_pool(name="xs", bufs=2) as x_pool,
            tc.tile_pool(name="res", bufs=2) as res_pool,
            tc.tile_pool(name="inpp", bufs=1) as inp_pool,
            tc.tile_pool(name="oacp", bufs=2) as oac_pool,
            tc.tile_pool(name="up", bufs=2) as u_pool,
            tc.tile_pool(name="sfp", bufs=3) as sf_pool,
            tc.tile_pool(name="srp", bufs=1) as sr_pool,
            tc.tile_pool(name="otp", bufs=2) as ot_pool,
            tc.tile_pool(name="w2p", bufs=2) as w2_pool,
            tc.tile_pool(name="stp", bufs=1) as st_pool,
            tc.tile_pool(name="abp", bufs=2) as ab_pool,
            tc.tile_pool(name="sc2", bufs=2) as sc2_pool,
            tc.tile_pool(name="ps1", bufs=3, space="PSUM") as ps1_pool,
            tc.tile_pool(name="ps2", bufs=3, space="PSUM") as ps2_pool,
            tc.tile_pool(name="pst", bufs=1, space="PSUM") as pst_pool,
            tc.tile_pool(name="dram", bufs=2, space="DRAM") as dram_pool,
        ):
            # ---- constants ----
            b1_sb = cst_pool.tile([P, M_TILES], f32, tag="b1")
            nc.sync.dma_start(b1_sb[:], b1_d.ap())
            gam_sb = cst_pool.tile([P, KC], f32, tag="gam")
            nc.scalar.dma_start(gam_sb[:], gam_d.ap())
            ones_sb = cst_pool.tile([P, 1], f16, tag="ones")
            nc.vector.memset(ones_sb[:], 1.0)
            eps_sb = cst_pool.tile([1, 1], f32, tag="eps")
            nc.vector.memset(eps_sb[:], 1e-5)
            c12_sb = cst_pool.tile([2, D], f16, tag="c12")
            nc.scalar.dma_start(c12_sb[:], c12_d.ap())
            w1_sb = w1_pool.tile([P, KC, 2 * P], f16, tag="w1")
            for q in range(KC):
                nc.sync.dma_start(w1_sb[:, q:q + 1, :], w1T_r[:, q:q + 1, :])
            w18_sb = w1_pool.tile([P, KD, 2, 4 * P], fp8, tag="w18")
            nc.sync.dma_start(w18_sb[:], w18_r)

            # state shared across the emit functions
            res = {}    # per-batch activation tiles
            ots = {}    # per-batch reshard output tiles
            sts = {}    # per-batch stat tiles
            scl = {}    # per-batch (a_full, ab2)

            def g1(b):
                """GEMM1 + activations for batch b."""
                lam_b = res_pool.tile([P, E, T], f16, tag="lam", name=f"lam{b}")
                inp_b = inp_pool.tile([P, E, T], f16, tag="inp", name=f"inp{b}")
                og_b = res_pool.tile([P, E, T], f16, tag="og", name=f"og{b}")
                res[b] = (lam_b, inp_b, og_b)
                dests = [(inp_b, AF.Silu), (inp_b, AF.Silu),
                         (og_b, AF.Sigmoid), (og_b, AF.Sigmoid),
                         (lam_b, AF.Sigmoid), (lam_b, AF.Sigmoid)]
                for nb in range(NB1):
                    tok0 = b * T + nb * NBLK
                    xt = x_pool.tile([P, KC, NBLK], f16, tag="xt")
                    nq = 16 if (b == 0 and nb == 0) else 4
                    for q in range(nq):
                        w_ = KC // nq
                        nc.sync.dma_start(
                            xt[:, w_ * q:w_ * (q + 1), :],
                            xT_r[:, w_ * q:w_ * (q + 1), tok0:tok0 + NBLK])
                    xt8 = x_pool.tile([P, KD, 2, NBLK], fp8, tag="xt8")
                    for q in range(2):
                        nc.sync.dma_start(
                            xt8[:, 4 * q:4 * (q + 1), :, :],
                            xT8_r[:, 4 * q:4 * (q + 1), :, tok0:tok0 + NBLK])
                    for m in range(M_TILES):
                        ps = ps1_pool.tile([P, NBLK], f32, tag="ps")
                        scale = 1.0
                        if m >= 2:
                            # og/fg: fp8 DoubleRow, weights prescaled x16
                            for kd in range(KD):
                                nc.tensor.matmul(
                                    ps[:],
                                    w18_sb[:, kd, :, (m - 2) * P:(m - 1) * P],
                                    xt8[:, kd, :, :],
                                    start=(kd == 0), stop=(kd == KD - 1),
                                    perf_mode=DR)
                            scale = 1.0 / 16.0
                        else:
                            for kc in range(KC):
                                nc.tensor.matmul(
                                    ps[:], w1_sb[:, kc, m * P:(m + 1) * P],
                                    xt[:, kc, :],
                                    start=(kc == 0), stop=(kc == KC - 1))
                        dest, func = dests[m]
                        dsl = dest[:, m % 2, nb * NBLK:(nb + 1) * NBLK]
                        nc.scalar.activation(dsl, ps[:], func,
                                             bias=b1_sb[:, m:m + 1],
                                             scale=scale)

            def scan_cc(b):
                """scans + o assembly (vector) + AllToAll round (gpsimd)."""
                lam_b, inp_b, og_b = res[b]
                oac_b = oac_pool.tile([P, E, T], f16, tag="oac", name=f"oac{b}")
                # per (e, d): u' = (lam-1)*inp (reused fwd+rev), fwd scan,
                # rev scan, A = s_f + s_r (overwrites s_f; u dies here);
                # then o_e = og0*A[0,e] + og1*A[1,e] so only 2 A-tiles live.
                for e in range(E):
                    A = {}
                    for d_ in range(E):
                        u = u_pool.tile([P, T], f16, tag="u",
                                        name=f"u{b}_{d_}{e}")
                        nc.vector.scalar_tensor_tensor(
                            u[:], lam_b[:, d_, :], 1.0, inp_b[:, e, :],
                            op0=SUB, op1=MUL)
                        s = sf_pool.tile([P, T], f16, tag="sf",
                                         name=f"sf{b}_{d_}{e}")
                        nc.vector.tensor_tensor_scan(
                            s[:], lam_b[:, d_, :], u[:], 0.0,
                            op0=MUL, op1=ADD)
                        sr = sr_pool.tile([P, T], f16, tag="sr")
                        nc.vector.tensor_tensor_scan(
                            sr[:, ::-1], lam_b[:, d_, ::-1],
                            u[:, ::-1], 0.0, op0=MUL, op1=ADD)
                        nc.vector.tensor_tensor(s[:], s[:], sr[:], ADD)
                        A[d_] = s
                    o_be = oac_b[:, e, :]
                    nc.vector.tensor_tensor(o_be, og_b[:, 0, :], A[0][:], MUL)
                    nc.vector.tensor_tensor(A[1][:], og_b[:, 1, :], A[1][:],
                                            MUL)
                    nc.vector.tensor_tensor(o_be, o_be, A[1][:], ADD)
                if debug_dump:
                    nc.gpsimd.dma_start(dbg_oac.ap()[b], oac_b[:])

                # AllToAll round (gpsimd queue)
                cc_in = dram_pool.tile([NCORES, P, E, NSEG], f16, tag="cc_in")
                cc_out = dram_pool.tile([NCORES, P, E, NSEG], f16, tag="cc_out")
                for j in range(NCORES):
                    nc.gpsimd.dma_start(
                        cc_in[j], oac_b[:, :, j * NSEG:(j + 1) * NSEG])
                nc.gpsimd.collective_compute(
                    "AllToAll", mybir.AluOpType.bypass,
                    replica_groups=[list(range(NCORES))],
                    ins=[cc_in.opt()], outs=[cc_out.opt()])
                cc_out_r = cc_out.rearrange("j p e t -> (j p e) t")
                ot = ot_pool.tile([P, KC, NSEG], f16, tag="ot", name=f"ot{b}")
                for kc in range(KC):
                    nc.gpsimd.dma_start(ot[:, kc, :],
                                        cc_out_r[kc * P:(kc + 1) * P, :])
                if debug_dump:
                    nc.gpsimd.dma_start(dbg_ot.ap()[b], ot[:])
                ots[b] = ot

            def stats_mm(b):
                """LN stat matmuls for batch b (tensor queue)."""
                ot = ots[b]
                st = st_pool.tile([1, 5, NSEG], f32, tag="st", name=f"st{b}")
                sts[b] = st
                SUM, SSQ = 0, 1
                for si in (SUM, SSQ):
                    pss = pst_pool.tile([1, NSEG], f32, tag="pst")
                    for kc in range(KC):
                        if si == SUM:
                            rhs = ot[:, kc, :]
                        else:
                            sq = sc2_pool.tile([P, NSEG], f16, tag="sq")
                            nc.scalar.square(sq[:], ot[:, kc, :])
                            rhs = sq[:]
                        nc.tensor.matmul(pss[:], ones_sb[:], rhs,
                                         start=(kc == 0), stop=(kc == KC - 1))
                    nc.vector.tensor_copy(out=st[:, si], in_=pss[:])

            def st_chain_scale(b):
                """stat scalar chain + a broadcast + ot prescale."""
                ot = ots[b]
                st = sts[b]
                # 5 slots, reusing dead ones: STD->SSQ slot, A->M2 slot, BB->SUM slot
                SUM, SSQ, MU, VAR, M2 = range(5)
                STD, A, BB_ = SSQ, M2, SUM
                nc.scalar.mul(st[:, MU], st[:, SUM], 1.0 / D)
                nc.vector.tensor_tensor(st[:, VAR], st[:, MU], st[:, MU], MUL)
                nc.scalar.mul(st[:, M2], st[:, SSQ], 1.0 / D)
                nc.vector.tensor_tensor(st[:, VAR], st[:, M2], st[:, VAR], SUB)
                nc.scalar.activation(st[:, STD], st[:, VAR], AF.Sqrt,
                                     bias=eps_sb[:])
                nc.vector.reciprocal(st[:, A], st[:, STD])       # rstd
                nc.vector.tensor_tensor(st[:, BB_], st[:, A], st[:, MU], MUL)
                nc.scalar.mul(st[:, A], st[:, A], -1.0)          # a = -rstd

                a_full = ab_pool.tile([P, NSEG], f32, tag="afull",
                                      name=f"af{b}")
                nc.gpsimd.partition_broadcast(a_full[:], st[:, A])
                ab2 = ab_pool.tile([2, NSEG], f16, tag="ab2", name=f"ab2{b}")
                nc.vector.memset(ab2[:], 1.0)
                nc.vector.tensor_copy(out=ab2[0:1, :], in_=st[:, BB_])
                # ot <- ot * gamma[p] * a[t]  (in place, f16)
                for kc in range(KC):
                    nc.vector.scalar_tensor_tensor(
                        ot[:, kc, :], ot[:, kc, :], gam_sb[:, kc:kc + 1],
                        a_full[:], op0=MUL, op1=MUL)
                scl[b] = (a_full, ab2)

            def g2(b):
                """GEMM2 with folded LN epilogue for batch b."""
                ot = ots[b]
                _, ab2 = scl[b]
                for oc in range(NOC):
                    ocs = slice(oc * OCB, (oc + 1) * OCB)
                    w2t = w2_pool.tile([P, KC, OCB], f16, tag="w2",
                                       name=f"w2_{b}_{oc}")
                    for q in range(4):
                        nc.sync.dma_start(
                            w2t[:, 4 * q:4 * (q + 1), :],
                            w2T_r[:, 4 * q:4 * (q + 1), ocs])
                    for tch in range(NTCH):
                        tsl = slice(tch * TCH, (tch + 1) * TCH)
                        ps2 = ps2_pool.tile([TCH, OCB], f32, tag="ps")
                        for kc in range(KC):
                            nc.tensor.matmul(
                                ps2[:], ot[:, kc, tsl],
                                w2t[:, kc, :],
                                start=(kc == 0), stop=False)
                        # + [b_t;1] @ [c1;c2]  (K=2 fold)
                        nc.tensor.matmul(ps2[:], ab2[:, tsl],
                                         c12_sb[:, ocs],
                                         start=False, stop=True)
                        for h_ in range(2):
                            hsl = slice(h_ * (OCB // 2), (h_ + 1) * (OCB // 2))
                            ob = sc2_pool.tile([TCH, OCB // 2], f32, tag="ob")
                            nc.scalar.copy(ob[:], ps2[:, hsl])
                            nc.scalar.dma_start(
                                out_d.ap()[b * NSEG + tch * TCH:
                                           b * NSEG + (tch + 1) * TCH,
                                           oc * OCB + h_ * (OCB // 2):
                                           oc * OCB + (h_ + 1) * (OCB // 2)],
                                ob[:])

            # ======= emission order (interleaves pass2 into pass1) =======
            g1(0); scan_cc(0)
            g1(1); scan_cc(1)
            g1(2); scan_cc(2)
            stats_mm(0); st_chain_scale(0)
            g1(3); scan_cc(3)
            g2(0)
            stats_mm(1); st_chain_scale(1)
            g2(1)
            stats_mm(2); st_chain_scale(2)
            g2(2)
            stats_mm(3); st_chain_scale(3)
            g2(3)

    nc.compile()
    return nc


def host_prep(x, W_in, b_in, gamma, beta, W_out, b_out, T=N_FULL):
    """Host-side input prep: fp16/fp8 casts, transposes, per-core W_in slices."""
    import ml_dtypes
    x = np.asarray(x)
    gamma = np.asarray(gamma, np.float32)
    beta = np.asarray(beta, np.float32)
    W_out = np.asarray(W_out, np.float32)
    b_out = np.asarray(b_out, np.float32)
    W_in = np.asarray(W_in, np.float32)
    b_in = np.asarray(b_in, np.float32)

    xf = np.ascontiguousarray(np.asarray(x, np.float32).transpose(2, 1, 0)
                              .reshape(D, B * T))
    xT = xf.astype(np.float16)
    xT8 = xf.astype(ml_dtypes.float8_e4m3fn)
    w2T = np.ascontiguousarray(W_out.T).astype(np.float16)
    gam = np.ascontiguousarray(gamma.reshape(KC, P).T)
    c12 = np.ascontiguousarray(
        np.stack([gamma @ W_out.T, beta @ W_out.T + b_out])).astype(np.float16)

    in_maps = []
    for c in range(NCORES):
        base = c * 2 * P
        rows = []
        for blk in range(3):                  # inp, og, fg
            for e in range(E):                # e0, e1 (or d0, d1 for fg)
                rows.append(blk * D + base + 2 * np.arange(P) + e)
        rows = np.concatenate(rows)           # (768,)
        w1T_c = np.ascontiguousarray(W_in[rows[:2 * P], :].T).astype(np.float16)
        w18_c = np.ascontiguousarray(16.0 * W_in[rows[2 * P:], :].T).astype(
            ml_dtypes.float8_e4m3fn)
        b1_c = np.ascontiguousarray(b_in[rows].reshape(M_TILES, P).T)
        in_maps.append({
            "xT": xT, "xT8": xT8, "w1T": w1T_c, "w18": w18_c, "b1": b1_c,
            "w2T": w2T, "gam": gam, "c12": c12,
        })
    return in_maps


def assemble_output(results, T=N_FULL):
    """Gather per-core [TOK_C, D] outputs into the full (N, B, D) array.

    Core i's local row (b*NSEG + n_loc) holds token (n = i*NSEG + n_loc, b).
    """
    NSEG = T // NCORES
    out = np.empty((T, B, D), np.float32)
    for i, res in enumerate(results):
        blk = res["out"].reshape(B, NSEG, D)
        for b in range(B):
            out[i * NSEG:(i + 1) * NSEG, b, :] = blk[b]
    return out


def kernel(x, W_in, b_in, gamma, beta, W_out, b_out):
    from concourse.bass_utils import run_bass_kernel_spmd

    key = N_FULL
    if key not in _BUILD_CACHE:
        _BUILD_CACHE[key] = build_program(N_FULL)
    nc = _BUILD_CACHE[key]
    in_maps = host_prep(x, W_in, b_in, gamma, beta, W_out, b_out)
    res = run_bass_kernel_spmd(nc, in_maps, core_ids=list(range(NCORES)))
    return assemble_output(res.results)


if __name__ == "__main__":
    import reference
    inputs = {k: np.asarray(v) for k, v in reference.setup_inputs().items()}
    expected = np.asarray(reference.reference(**inputs))
    actual = kernel(**inputs)
    err = np.abs(actual - expected)
    rel = np.linalg.norm(actual - expected) / np.linalg.norm(expected)
    print("max abs err:", err.max(), "rel fro err:", rel)


# revision 13
# speedup vs baseline: 1.0170x; 1.0170x over previous
"""Bass/Trainium2 kernel for nn_BiHgru2_1d (bidirectional HGRU block), 8-core SPMD.

Math (reference):
    feat = x @ W_in.T + b_in                    # (N,B,3D)
    inp, og, fg = split(feat); inp=silu(inp); og=sigmoid(og); lam=sigmoid(fg)
    u[h,d,e] = (1-lam[h,d]) * inp[h,e];  lam_f[h,d,e] = lam[h,d]
    s = fwd_scan(lam_f, u) + rev_scan(lam_f, u)         # h_t = lam_t h_{t-1} + u_t
    o[h,e] = sum_d s[h,d,e]*og[h,d]; o = LN(o)*gamma+beta; out = o @ W_out.T + b_out

Sharding: 8-way tensor parallel over heads (128 heads/core). Each core:
  GEMM1 (x full; inp path f16, og+fg paths fp8e4m3 DoubleRow) -> activations ->
  per-(b,d,e) tensor_tensor_scan fwd+rev (u reused between directions) ->
  o assembly -> per-batch AllToAll (reshard channel->token) -> LayerNorm fully
  folded into GEMM2 (ot prescaled by gamma*(-rstd), b_t*c1+c2 added as a K=2
  row of the same PSUM accumulation) -> each core writes tokens
  (all b, n in [256i, 256(i+1))) of the output.

Sign trick: we compute u' = (lam-1)*inp = -u, so s' = -s, o' = -o.  With
a_t = -rstd_t and b_t = rstd_t*mu'_t the GEMM2 input ot*gamma*a_t and the
K=2 fold [b_t;1]@[c1;c2] give the exact LN(o)@W2T + c2 despite the sign.

Pass2 is interleaved into pass1 on the tensor queue to keep the PE array
warm (HAM clock gate) and hide the collective latency.
"""

import sys

for _p in ("/opt/trn_rl_repo",):
    if _p not in sys.path:
        sys.path.insert(0, _p)

import numpy as np

# ---- problem constants (hardcoded per contract) ----
N_FULL, B, D = 2048, 4, 2048
E = 2
H = D // E                      # 1024 heads
NCORES = 8
P = 128                         # partitions
HC = H // NCORES                # 128 heads per core
KC = D // P                     # 16 k-chunks
M_TILES = 6                     # [inp e0, inp e1, og e0, og e1, fg d0, fg d1]

_BUILD_CACHE = {}


def build_program(T=N_FULL, num_devices=NCORES, debug_dump=False):
    """Build the SPMD Bass program (same program on every core)."""
    import concourse.bass as bass
    import concourse.mybir as mybir
    import concourse.tile as tile
    from concourse import bacc

    f16 = mybir.dt.float16
    f32 = mybir.dt.float32
    fp8 = mybir.dt.float8e4
    MUL = mybir.AluOpType.mult
    ADD = mybir.AluOpType.add
    SUB = mybir.AluOpType.subtract
    AF = mybir.ActivationFunctionType
    DR = mybir.MatmulPerfMode.DoubleRow

    NSEG = T // NCORES           # per-core seq positions per batch (256)
    TOK_C = B * NSEG             # tokens per core after reshard (1024)
    NBLK = min(512, T)           # GEMM1 token-block size (per batch)
    NB1 = T // NBLK              # token blocks per batch (4)
    TCH = min(P, NSEG)           # GEMM2 token-chunk (output partition dim)
    NTCH = NSEG // TCH           # token chunks per round (2)
    OCB = 512                    # GEMM2 out-col block
    NOC = D // OCB
    KD = KC // 2                 # DoubleRow k-chunk pairs (8)
    assert T % (NCORES * TCH) == 0 and T % NBLK == 0

    nc = bacc.Bacc("TRN2", target_bir_lowering=False, debug=False,
                   num_devices=num_devices)

    # ---- per-core DRAM parameters ----
    xT_d = nc.dram_tensor("xT", [D, B * T], f16, kind="ExternalInput")
    xT8_d = nc.dram_tensor("xT8", [D, B * T], fp8, kind="ExternalInput")
    w1T_d = nc.dram_tensor("w1T", [D, 2 * P], f16, kind="ExternalInput")
    w18_d = nc.dram_tensor("w18", [D, 4 * P], fp8, kind="ExternalInput")
    b1_d = nc.dram_tensor("b1", [P, M_TILES], f32, kind="ExternalInput")
    w2T_d = nc.dram_tensor("w2T", [D, D], f16, kind="ExternalInput")
    c12_d = nc.dram_tensor("c12", [2, D], f16, kind="ExternalInput")
    out_d = nc.dram_tensor("out", [TOK_C, D], f32, kind="ExternalOutput")
    if debug_dump:
        dbg_oac = nc.dram_tensor("dbg_oac", [B, P, E, T], f16, kind="ExternalOutput")
        dbg_ot = nc.dram_tensor("dbg_ot", [B, P, KC, NSEG], f16, kind="ExternalOutput")

    xT_r = xT_d.ap().rearrange("(kc p) t -> p kc t", p=P)
    xT8_r = xT8_d.ap().rearrange("(kd ko p) t -> p kd ko t", p=P, ko=2)
    w1T_r = w1T_d.ap().rearrange("(kc p) m -> p kc m", p=P)
    w18_r = w18_d.ap().rearrange("(kd ko p) m -> p kd ko m", p=P, ko=2)
    w2T_r = w2T_d.ap().rearrange("(kc p) o -> p kc o", p=P)

    with tile.TileContext(nc) as tc:
        with (
            tc.tile_pool(name="cst", bufs=1) as cst_pool,
            tc.tile_pool(name="w1p", bufs=1) as w1_pool,
            tc.tile_pool(name="xs", bufs=2) as x_pool,
            tc.tile_pool(name="res", bufs=2) as res_pool,
            tc.tile_pool(name="inpp", bufs=1) as inp_pool,
            tc.tile_pool(name="oacp", bufs=2) as oac_pool,
            tc.tile_pool(name="up", bufs=2) as u_pool,
            tc.tile_pool(name="sfp", bufs=3) as sf_pool,
            tc.tile_pool(name="srp", bufs=1) as sr_pool,
            tc.tile_pool(name="otp", bufs=2) as ot_pool,
            tc.tile_pool(name="w2p", bufs=2) as w2_pool,
            tc.tile_pool(name="stp", bufs=1) as st_pool,
            tc.tile_pool(name="abp", bufs=2) as ab_pool,
            tc.tile_pool(name="sc2", bufs=2) as sc2_pool,
            tc.tile_pool(name="ps1", bufs=3, space="PSUM") as ps1_pool,
            tc.tile_pool(name="ps2", bufs=3, space="PSUM") as ps2_pool,
            tc.tile_pool(name="pst", bufs=1, space="PSUM") as pst_pool,
            tc.tile_pool(name="dram", bufs=2, space="DRAM") as dram_pool,
        ):
            # ---- constants ----
            b1_sb = cst_pool.tile([P, M_TILES], f32, tag="b1")
            nc.sync.dma_start(b1_sb[:], b1_d.ap())
            ones_sb = cst_pool.tile([P, 1], f16, tag="ones")
            nc.vector.memset(ones_sb[:], 1.0)
            eps_sb = cst_pool.tile([1, 1], f32, tag="eps")
            nc.vector.memset(eps_sb[:], 1e-5)
            c12_sb = cst_pool.tile([2, D], f16, tag="c12")
            nc.scalar.dma_start(c12_sb[:], c12_d.ap())
            w1_sb = w1_pool.tile([P, KC, 2 * P], f16, tag="w1")
            for q in range(KC):
                nc.sync.dma_start(w1_sb[:, q:q + 1, :], w1T_r[:, q:q + 1, :])
            w18_sb = w1_pool.tile([P, KD, 2, 4 * P], fp8, tag="w18")
            nc.sync.dma_start(w18_sb[:], w18_r)

            # state shared across the emit functions
            res = {}    # per-batch activation tiles
            ots = {}    # per-batch reshard output tiles
            sts = {}    # per-batch stat tiles
            scl = {}    # per-batch (a_full, ab2)

            def g1(b):
                """GEMM1 + activations for batch b."""
                lam_b = res_pool.tile([P, E, T], f16, tag="lam", name=f"lam{b}")
                inp_b = inp_pool.tile([P, E, T], f16, tag="inp", name=f"inp{b}")
                og_b = res_pool.tile([P, E, T], f16, tag="og", name=f"og{b}")
                res[b] = (lam_b, inp_b, og_b)
                dests = [(inp_b, AF.Silu), (inp_b, AF.Silu),
                         (og_b, AF.Sigmoid), (og_b, AF.Sigmoid),
                         (lam_b, AF.Sigmoid), (lam_b, AF.Sigmoid)]
                for nb in range(NB1):
                    tok0 = b * T + nb * NBLK
                    xt = x_pool.tile([P, KC, NBLK], f16, tag="xt")
                    nq = 16 if (b == 0 and nb == 0) else 4
                    for q in range(nq):
                        w_ = KC // nq
                        nc.sync.dma_start(
                            xt[:, w_ * q:w_ * (q + 1), :],
                            xT_r[:, w_ * q:w_ * (q + 1), tok0:tok0 + NBLK])
                    xt8 = x_pool.tile([P, KD, 2, NBLK], fp8, tag="xt8")
                    for q in range(2):
                        nc.sync.dma_start(
                            xt8[:, 4 * q:4 * (q + 1), :, :],
                            xT8_r[:, 4 * q:4 * (q + 1), :, tok0:tok0 + NBLK])
                    for m in range(M_TILES):
                        ps = ps1_pool.tile([P, NBLK], f32, tag="ps")
                        scale = 1.0
                        if m >= 2:
                            # og/fg: fp8 DoubleRow, weights prescaled x16
                            for kd in range(KD):
                                nc.tensor.matmul(
                                    ps[:],
                                    w18_sb[:, kd, :, (m - 2) * P:(m - 1) * P],
                                    xt8[:, kd, :, :],
                                    start=(kd == 0), stop=(kd == KD - 1),
                                    perf_mode=DR)
                            scale = 1.0 / 16.0
                        else:
                            for kc in range(KC):
                                nc.tensor.matmul(
                                    ps[:], w1_sb[:, kc, m * P:(m + 1) * P],
                                    xt[:, kc, :],
                                    start=(kc == 0), stop=(kc == KC - 1))
                        dest, func = dests[m]
                        dsl = dest[:, m % 2, nb * NBLK:(nb + 1) * NBLK]
                        nc.scalar.activation(dsl, ps[:], func,
                                             bias=b1_sb[:, m:m + 1],
                                             scale=scale)

            def scan_cc(b):
                """scans + o assembly (vector) + AllToAll round (gpsimd)."""
                lam_b, inp_b, og_b = res[b]
                oac_b = oac_pool.tile([P, E, T], f16, tag="oac", name=f"oac{b}")
                # per (e, d): u' = (lam-1)*inp (reused fwd+rev), fwd scan,
                # rev scan, A = s_f + s_r (overwrites s_f; u dies here);
                # then o_e = og0*A[0,e] + og1*A[1,e] so only 2 A-tiles live.
                for e in range(E):
                    A = {}
                    for d_ in range(E):
                        u = u_pool.tile([P, T], f16, tag="u",
                                        name=f"u{b}_{d_}{e}")
                        nc.vector.scalar_tensor_tensor(
                            u[:], lam_b[:, d_, :], 1.0, inp_b[:, e, :],
                            op0=SUB, op1=MUL)
                        s = sf_pool.tile([P, T], f16, tag="sf",
                                         name=f"sf{b}_{d_}{e}")
                        nc.vector.tensor_tensor_scan(
                            s[:], lam_b[:, d_, :], u[:], 0.0,
                            op0=MUL, op1=ADD)
                        sr = sr_pool.tile([P, T], f16, tag="sr")
                        nc.vector.tensor_tensor_scan(
                            sr[:, ::-1], lam_b[:, d_, ::-1],
                            u[:, ::-1], 0.0, op0=MUL, op1=ADD)
                        nc.vector.tensor_tensor(s[:], s[:], sr[:], ADD)
                        A[d_] = s
                    o_be = oac_b[:, e, :]
                    nc.vector.tensor_tensor(o_be, og_b[:, 0, :], A[0][:], MUL)
                    nc.vector.tensor_tensor(A[1][:], og_b[:, 1, :], A[1][:],
                                            MUL)
                    nc.vector.tensor_tensor(o_be, o_be, A[1][:], ADD)
                if debug_dump:
                    nc.gpsimd.dma_start(dbg_oac.ap()[b], oac_b[:])

                # AllToAll round (gpsimd queue)
                cc_in = dram_pool.tile([NCORES, P, E, NSEG], f16, tag="cc_in")
                cc_out = dram_pool.tile([NCORES, P, E, NSEG], f16, tag="cc_out")
                for j in range(NCORES):
                    nc.gpsimd.dma_start(
                        cc_in[j], oac_b[:, :, j * NSEG:(j + 1) * NSEG])
                nc.gpsimd.collective_compute(
                    "AllToAll", mybir.AluOpType.bypass,
                    replica_groups=[list(range(NCORES))],
                    ins=[cc_in.opt()], outs=[cc_out.opt()])
                cc_out_r = cc_out.rearrange("j p e t -> (j p e) t")
                ot = ot_pool.tile([P, KC, NSEG], f16, tag="ot", name=f"ot{b}")
                for kc in range(KC):
                    nc.gpsimd.dma_start(ot[:, kc, :],
                                        cc_out_r[kc * P:(kc + 1) * P, :])
                if debug_dump:
                    nc.gpsimd.dma_start(dbg_ot.ap()[b], ot[:])
                ots[b] = ot

            def stats_mm(b):
                """LN stat matmuls for batch b (tensor queue)."""
                ot = ots[b]
                st = st_pool.tile([1, 5, NSEG], f32, tag="st", name=f"st{b}")
                sts[b] = st
                SUM, SSQ = 0, 1
                for si in (SUM, SSQ):
                    pss = pst_pool.tile([1, NSEG], f32, tag="pst")
                    for kc in range(KC):
                        if si == SUM:
                            rhs = ot[:, kc, :]
                        else:
                            sq = sc2_pool.tile([P, NSEG], f16, tag="sq")
                            nc.scalar.square(sq[:], ot[:, kc, :])
                            rhs = sq[:]
                        nc.tensor.matmul(pss[:], ones_sb[:], rhs,
                                         start=(kc == 0), stop=(kc == KC - 1))
                    nc.vector.tensor_copy(out=st[:, si], in_=pss[:])

            def st_chain_scale(b):
                """stat scalar chain + a broadcast + ot prescale."""
                ot = ots[b]
                st = sts[b]
                # 5 slots, reusing dead ones: STD->SSQ slot, A->M2 slot, BB->SUM slot
                SUM, SSQ, MU, VAR, M2 = range(5)
                STD, A, BB_ = SSQ, M2, SUM
                nc.scalar.mul(st[:, MU], st[:, SUM], 1.0 / D)
                nc.vector.tensor_tensor(st[:, VAR], st[:, MU], st[:, MU], MUL)
                nc.scalar.mul(st[:, M2], st[:, SSQ], 1.0 / D)
                nc.vector.tensor_tensor(st[:, VAR], st[:, M2], st[:, VAR], SUB)
                nc.scalar.activation(st[:, STD], st[:, VAR], AF.Sqrt,
                                     bias=eps_sb[:])
                nc.vector.reciprocal(st[:, A], st[:, STD])       # rstd
                nc.vector.tensor_tensor(st[:, BB_], st[:, A], st[:, MU], MUL)
                nc.scalar.mul(st[:, A], st[:, A], -1.0)          # a = -rstd

                a16 = ab_pool.tile([1, NSEG], f16, tag="a16", name=f"a16{b}")
                nc.vector.tensor_copy(out=a16[:], in_=st[:, A])
                a_full = ab_pool.tile([P, NSEG], f16, tag="afull",
                                      name=f"af{b}")
                nc.gpsimd.partition_broadcast(a_full[:], a16[:])
                ab2 = ab_pool.tile([2, NSEG], f16, tag="ab2", name=f"ab2{b}")
                nc.vector.memset(ab2[:], 1.0)
                nc.vector.tensor_copy(out=ab2[0:1, :], in_=st[:, BB_])
                # ot <- ot * a[t]  (gamma is folded into w2T rows on the
                # host; in place, f16, gpsimd so the DVE stays free for
                # scans; kc-ordered so G2 MMs can chase)
                for kc in range(KC):
                    nc.gpsimd.tensor_tensor(
                        ot[:, kc, :], ot[:, kc, :], a_full[:], MUL)
                scl[b] = (a_full, ab2)

            def g2(b):
                """GEMM2 with folded LN epilogue for batch b."""
                ot = ots[b]
                _, ab2 = scl[b]
                for oc in range(NOC):
                    ocs = slice(oc * OCB, (oc + 1) * OCB)
                    w2t = w2_pool.tile([P, KC, OCB], f16, tag="w2",
                                       name=f"w2_{b}_{oc}")
                    for q in range(4):
                        nc.sync.dma_start(
                            w2t[:, 4 * q:4 * (q + 1), :],
                            w2T_r[:, 4 * q:4 * (q + 1), ocs])
                    for tch in range(NTCH):
                        tsl = slice(tch * TCH, (tch + 1) * TCH)
                        ps2 = ps2_pool.tile([TCH, OCB], f32, tag="ps")
                        for kc in range(KC):
                            nc.tensor.matmul(
                                ps2[:], ot[:, kc, tsl],
                                w2t[:, kc, :],
                                start=(kc == 0), stop=False)
                        # + [b_t;1] @ [c1;c2]  (K=2 fold)
                        nc.tensor.matmul(ps2[:], ab2[:, tsl],
                                         c12_sb[:, ocs],
                                         start=False, stop=True)
                        for h_ in range(2):
                            hsl = slice(h_ * (OCB // 2), (h_ + 1) * (OCB // 2))
                            ob = sc2_pool.tile([TCH, OCB // 2], f32, tag="ob")
                            nc.scalar.copy(ob[:], ps2[:, hsl])
                            nc.scalar.dma_start(
                                out_d.ap()[b * NSEG + tch * TCH:
                                           b * NSEG + (tch + 1) * TCH,
                                           oc * OCB + h_ * (OCB // 2):
                                           oc * OCB + (h_ + 1) * (OCB // 2)],
                                ob[:])

            # ======= emission order =======
            # tensor queue: all G1 first (dense, keeps PE warm), then stats
            # pulled as early as their collective allows, G2 following so
            # each batch's stat chain + scales complete during the previous
            # batch's G2.
            g1(0); scan_cc(0)
            g1(1); scan_cc(1)
            g1(2); scan_cc(2)
            g1(3); scan_cc(3)
            stats_mm(0); st_chain_scale(0)
            stats_mm(1); st_chain_scale(1)
            g2(0)
            g2(1)
            stats_mm(2); st_chain_scale(2)
            g2(2)
            stats_mm(3); st_chain_scale(3)
            g2(3)

    nc.compile()
    return nc


def host_prep(x, W_in, b_in, gamma, beta, W_out, b_out, T=N_FULL):
    """Host-side input prep: fp16/fp8 casts, transposes, per-core W_in slices."""
    import ml_dtypes
    x = np.asarray(x)
    gamma = np.asarray(gamma, np.float32)
    beta = np.asarray(beta, np.float32)
    W_out = np.asarray(W_out, np.float32)
    b_out = np.asarray(b_out, np.float32)
    W_in = np.asarray(W_in, np.float32)
    b_in = np.asarray(b_in, np.float32)

    xf = np.ascontiguousarray(np.asarray(x, np.float32).transpose(2, 1, 0)
                              .reshape(D, B * T))
    xT = xf.astype(np.float16)
    xT8 = xf.astype(ml_dtypes.float8_e4m3fn)
    w2T = np.ascontiguousarray((W_out * gamma[None, :]).T).astype(np.float16)
    c12 = np.ascontiguousarray(
        np.stack([gamma @ W_out.T, beta @ W_out.T + b_out])).astype(np.float16)

    in_maps = []
    for c in range(NCORES):
        base = c * 2 * P
        rows = []
        for blk in range(3):                  # inp, og, fg
            for e in range(E):                # e0, e1 (or d0, d1 for fg)
                rows.append(blk * D + base + 2 * np.arange(P) + e)
        rows = np.concatenate(rows)           # (768,)
        w1T_c = np.ascontiguousarray(W_in[rows[:2 * P], :].T).astype(np.float16)
        w18_c = np.ascontiguousarray(16.0 * W_in[rows[2 * P:], :].T).astype(
            ml_dtypes.float8_e4m3fn)
        b1_c = np.ascontiguousarray(b_in[rows].reshape(M_TILES, P).T)
        in_maps.append({
            "xT": xT, "xT8": xT8, "w1T": w1T_c, "w18": w18_c, "b1": b1_c,
            "w2T": w2T, "c12": c12,
        })
    return in_maps


def assemble_output(results, T=N_FULL):
    """Gather per-core [TOK_C, D] outputs into the full (N, B, D) array.

    Core i's local row (b*NSEG + n_loc) holds token (n = i*NSEG + n_loc, b).
    """
    NSEG = T // NCORES
    out = np.empty((T, B, D), np.float32)
    for i, res in enumerate(results):
        blk = res["out"].reshape(B, NSEG, D)
        for b in range(B):
            out[i * NSEG:(i + 1) * NSEG, b, :] = blk[b]
    return out


def kernel(x, W_in, b_in, gamma, beta, W_out, b_out):
    from concourse.bass_utils import run_bass_kernel_spmd

    key = N_FULL
    if key not in _BUILD_CACHE:
        _BUILD_CACHE[key] = build_program(N_FULL)
    nc = _BUILD_CACHE[key]
    in_maps = host_prep(x, W_in, b_in, gamma, beta, W_out, b_out)
    res = run_bass_kernel_spmd(nc, in_maps, core_ids=list(range(NCORES)))
    return assemble_output(res.results)


if __name__ == "__main__":
    import reference
    inputs = {k: np.asarray(v) for k, v in reference.setup_inputs().items()}
    expected = np.asarray(reference.reference(**inputs))
    actual = kernel(**inputs)
    err = np.abs(actual - expected)
    rel = np.linalg.norm(actual - expected) / np.linalg.norm(expected)
    print("max abs err:", err.max(), "rel fro err:", rel)
